# revision 1
# baseline (speedup 1.0000x reference)
"""DGCNN-style kernel for Trainium2 (8 NeuronCores, data-parallel over batch).

Per core: one batch sample, both branches (x, y).
Pipeline per branch:
  1. exact pairwise -d matrix (PE K=2 f32 matmul, bitwise-matching the
     neuron-executed reference einsum) assembled as A = 2*dot - (sq_n + sq_j)
  2. per-row top-32 (chunked max8/max_index/match_replace cascade, stable ties)
  3. rank-weight matrices W1/W2 built by GPSIMD local_scatter (fp16)
  4. per-point features H1 (48ch) / H2 (96ch) via PE (BN folded on host)
  5. X1 = W1 @ H1, X2 = W2 @ H2 via PE (W blocks transposed on PE)
  6. cross-branch max-pool, 288->96 conv (pooled part folded into bias),
     GroupNorm(12), relu, transpose, row L2-normalize.
"""
import sys

sys.path.insert(0, '/opt/trn_rl_repo')
sys.path.insert(0, '/opt/pypackages')

import numpy as np
import concourse.bacc as bacc
import concourse.mybir as mybir
from concourse.tile import TileContext
from concourse.bass_utils import run_bass_kernel_spmd

N = 2048
K = 32
NT = N // 128          # 16 n-tiles
NCH = 8                # chunks per row for the cascade
CHW = N // NCH         # 128 chunk width
NCAND = NCH * 16       # 256 candidates per row
BN_EPS = 1e-5
GN_EPS = 1e-5
NEG = -1.0e9

f32 = mybir.dt.float32
f16 = mybir.dt.float16
u16 = mybir.dt.uint16
i16 = mybir.dt.int16
Alu = mybir.AluOpType
Act = mybir.ActivationFunctionType
AxX = mybir.AxisListType.X

_CACHED = {}


def _branch_phase12(nc, sb, sbd, ps, pts, consts, tag):
    """Load pts, build rows3/rows2/sq, H1e/H2T/Hcat. Returns dict of tiles."""
    t = {}
    flat = sb.tile([1, 2 * N], f32, tag="flat")
    nc.sync.dma_start(out=flat[:], in_=pts.rearrange("(a n) c -> a (n c)", a=1))
    xv = flat[0:1, :].rearrange("1 (n c) -> 1 n c", c=2)[:, :, 0]
    yv = flat[0:1, :].rearrange("1 (n c) -> 1 n c", c=2)[:, :, 1]


    # rows3 = [x; y; ones] via PE partition placement
    rows3 = sbd.tile([3, N], f32, tag="rows3")
    e01, e10 = consts['e01'], consts['e10']
    for c in range(4):
        sl = slice(c * 512, (c + 1) * 512)
        pr = ps.tile([2, 512], f32, tag="big")
        nc.tensor.matmul(pr, e01, xv[:, sl], start=True, stop=False)
        nc.tensor.matmul(pr, e10, yv[:, sl], start=False, stop=True)
        nc.vector.tensor_copy(rows3[0:2, sl], pr)
    nc.sync.dma_start(out=rows3[2:3, :], in_=consts['ones_dram'][0:1, :])
    rows2 = sbd.tile([2, N], f32, tag="rows2")
    nc.vector.tensor_scalar_mul(rows2, rows3[0:2, :], 2.0)

    # coords in column layout -> sq columns [128, NT] directly
    ccols = sbd.tile([128, NT, 2], f32, tag="ccols")
    nc.sync.dma_start(out=ccols[:], in_=pts.rearrange("(t p) c -> p t c", p=128))
    sqcols = sbd.tile([128, NT], f32, tag="sqcols")
    tc2 = sb.tile([128, NT], f32, tag="tc2")
    nc.vector.tensor_mul(sqcols, ccols[:, :, 0], ccols[:, :, 0])
    nc.vector.tensor_mul(tc2, ccols[:, :, 1], ccols[:, :, 1])
    nc.vector.tensor_add(sqcols, sqcols, tc2)

    # sq row in DRAM via PE transpose of sqcols, then broadcast to [128, N]
    pqt = ps.tile([NT, 128], f32, tag="sml")
    nc.tensor.transpose(pqt, sqcols, consts['id_f32'][0:128, 0:128])
    sqt = sb.tile([NT, 128], f32, tag="sqt")
    nc.vector.tensor_copy(sqt, pqt)
    sq_dram = nc.dram_tensor(f"sq_dram_{tag}", [1, N], f32)
    nc.sync.dma_start(out=sq_dram[0:1, :].rearrange("a (t p) -> (a t) p", p=128),
                      in_=sqt[:])
    sqb = sbd.tile([128, N], f32, tag="sqb")
    nc.sync.dma_start(out=sqb[:], in_=sq_dram[0:1, :].to_broadcast([128, N]))

    # H1e [49, N] f32 : relu(A1 p + c1) rows + ones row
    h1e = sb.tile([49, N], f32, tag="bigW1")  # shares slot with w1
    for c in range(4):
        sl = slice(c * 512, (c + 1) * 512)
        ph = ps.tile([48, 512], f32, tag="big")
        nc.tensor.matmul(ph, consts['lhs3'], rows3[:, sl], start=True, stop=True)
        nc.scalar.activation(h1e[0:48, sl], ph, Act.Relu)
    nc.sync.dma_start(out=h1e[48:49, :], in_=consts['ones_dram'][0:1, :])

    # H2T [96, N] f32 : relu(A2 H1 + c2)
    h2t = sb.tile([96, N], f32, tag="bigW2")  # shares slot with w2
    for c in range(4):
        sl = slice(c * 512, (c + 1) * 512)
        ph = ps.tile([96, 512], f32, tag="big")
        nc.tensor.matmul(ph, consts['lhs49'], h1e[:, sl], start=True, stop=True)
        nc.scalar.activation(h2t[:, sl], ph, Act.Relu)

    # Hcat [128, NT, 144] fp16 (row-major per j-block) via PE transposes
    hcat = sbd.tile([128, NT, 144], f16, tag="hcat")
    for jb in range(NT):
        jsl = slice(jb * 128, (jb + 1) * 128)
        p1 = ps.tile([128, 48], f32, tag="sml")
        nc.tensor.transpose(p1, h1e[0:48, jsl], consts['id_f32'][0:48, 0:48])
        nc.scalar.activation(hcat[:, jb, 0:48], p1, Act.Copy)
        p2 = ps.tile([128, 96], f32, tag="sml")
        nc.tensor.transpose(p2, h2t[:, jsl], consts['id_f32'][0:96, 0:96])
        nc.scalar.activation(hcat[:, jb, 48:144], p2, Act.Copy)

    t.update(rows2=rows2, rows3=rows3, sqb=sqb, sqcols=sqcols, hcat=hcat)
    return t


def _branch_main(nc, sb, sbd, ps, psb, br, consts, x1t, x2t):
    """Per n-tile: d-matrix, cascade, scatter, combine.

    Software-pipelined: tile tt's A-phase (PE matmuls + ACT s-builds + DVE
    subs) is emitted BEFORE tile tt-1's combine so the ACT FIFO is not
    blocked behind the weight-transpose copies (which wait on GPSIMD
    scatters), keeping the VectorE cascade stream fed.
    """
    rows2, rows3, sqb, sqcols, hcat = (br['rows2'], br['rows3'], br['sqb'],
                                       br['sqcols'], br['hcat'])

    def a_phase(tt):
        nsl = slice(tt * 128, (tt + 1) * 128)
        # ---- A = 2*dot - (sq_n + sq_j), bitwise == -(reference d) ----
        a = sbd.tile([128, N], f32, tag="bigA")
        for h in range(2):
            hsl = slice(h * 1024, (h + 1) * 1024)
            pt = ps.tile([128, 1024], f32, tag="big")
            for c in range(2):
                sl = slice(h * 1024 + c * 512, h * 1024 + (c + 1) * 512)
                nc.tensor.matmul(pt[:, c * 512:(c + 1) * 512],
                                 rows2[:, nsl], rows3[0:2, sl],
                                 start=True, stop=True)
            s = sbd.tile([128, 1024], f32, tag="s")
            nc.scalar.activation(s, sqb[:, hsl], Act.Relu,
                                 bias=sqcols[:, tt:tt + 1], scale=1.0)
            ac = sbd.tile([128, 1024], f32, tag="ac")
            nc.scalar.activation(ac, pt, Act.Copy)
            nc.gpsimd.tensor_sub(a[:, hsl], ac, s)
        return a

    def select_phase(a):
        # ---- chunked top-32 cascade ----
        cv = sbd.tile([128, NCAND], f32, tag="cv")
        ci = sbd.tile([128, NCAND], u16, tag="ci")
        for c in range(NCH):
            asl = a[:, c * CHW:(c + 1) * CHW]
            s0, s1 = slice(c * 16, c * 16 + 8), slice(c * 16 + 8, c * 16 + 16)
            nc.vector.max(cv[:, s0], asl)
            nc.vector.max_index(ci[:, s0], cv[:, s0], asl)
            nc.vector.match_replace(asl, in_to_replace=cv[:, s0],
                                    in_values=asl, imm_value=NEG)
            nc.vector.max(cv[:, s1], asl)
            nc.vector.max_index(ci[:, s1], cv[:, s1], asl)
        # merge: top-32 of the candidates (values + candidate positions)
        mm = sbd.tile([128, K], f32, tag="mm")
        pp = sbd.tile([128, K], u16, tag="pp")
        for r in range(4):
            rsl = slice(r * 8, r * 8 + 8)
            nc.vector.max(mm[:, rsl], cv)
            nc.vector.max_index(pp[:, rsl], mm[:, rsl], cv)
            if r < 3:
                nc.vector.match_replace(cv, in_to_replace=mm[:, rsl],
                                        in_values=cv, imm_value=NEG)

        # ---- global index arrays for the scatter (all int16, 2x mode) ----
        cig = sbd.tile([128, NCAND], i16, tag="cig")
        nc.vector.tensor_add(cig, ci[:].bitcast(i16), consts['offs16'])
        msk = sbd.tile([128, NCAND], i16, tag="msk")
        nc.vector.tensor_scalar(msk, cig, 1024.0, scalar2=None, op0=Alu.is_ge)
        lo_i = sbd.tile([128, NCAND], i16, tag="lo_i")
        nc.vector.scalar_tensor_tensor(lo_i, msk, -3000.0, cig,
                                       op0=Alu.mult, op1=Alu.add)
        hi_i = sbd.tile([128, NCAND], i16, tag="hi_i")
        nc.vector.tensor_scalar_sub(hi_i, cig, 1024.0)

        # ---- weight matrices via local_scatter ----
        cw1 = sbd.tile([128, NCAND], f16, tag="cw1")
        cw2 = sbd.tile([128, NCAND], f16, tag="cw2")
        ppi = pp[:].bitcast(i16)
        nc.gpsimd.local_scatter(cw1, consts['m1w_row'], ppi,
                                channels=128, num_elems=NCAND, num_idxs=K)
        nc.gpsimd.local_scatter(cw2, consts['m2w_row'], ppi,
                                channels=128, num_elems=NCAND, num_idxs=K)
        w1 = sb.tile([128, N], f16, tag="bigW1")
        w2 = sb.tile([128, N], f16, tag="bigW2")
        for w, cw in ((w1, cw1), (w2, cw2)):
            nc.gpsimd.local_scatter(w[:, 0:1024], cw, lo_i[:],
                                    channels=128, num_elems=1024, num_idxs=NCAND)
            nc.gpsimd.local_scatter(w[:, 1024:2048], cw, hi_i[:],
                                    channels=128, num_elems=1024, num_idxs=NCAND)
        return w1, w2

    def combine_phase(tt, w1, w2):
        nsl = slice(tt * 128, (tt + 1) * 128)
        # ---- combine: X1^T[48, nsl] = sum_j H1[j,:]^T W1^T[j, n] ----
        px1 = psb.tile([48, 128], f32, tag="px1")
        px2 = psb.tile([96, 128], f32, tag="px2")
        for jb in range(NT):
            jsl = slice(jb * 128, (jb + 1) * 128)
            pw1 = ps.tile([128, 128], f16, tag="sml")
            nc.tensor.transpose(pw1, w1[:, jsl], consts['id_f16'])
            wt1 = sbd.tile([128, 128], f16, tag="wt1")
            nc.scalar.activation(wt1, pw1, Act.Copy)
            pw2 = ps.tile([128, 128], f16, tag="sml")
            nc.tensor.transpose(pw2, w2[:, jsl], consts['id_f16'])
            wt2 = sbd.tile([128, 128], f16, tag="wt2")
            nc.scalar.activation(wt2, pw2, Act.Copy)
            nc.tensor.matmul(px1, hcat[:, jb, 0:48], wt1,
                             start=(jb == 0), stop=(jb == NT - 1))
            nc.tensor.matmul(px2, hcat[:, jb, 48:144], wt2,
                             start=(jb == 0), stop=(jb == NT - 1))
        nc.scalar.activation(x1t[:, nsl], px1, Act.Identity,
                             bias=consts['m1b_col'][:, 0:1], scale=1.0)
        nc.scalar.activation(x2t[:, nsl], px2, Act.Identity,
                             bias=consts['m2b_col'][:, 0:1], scale=1.0)

    prev = None
    for tt in range(NT):
        a = a_phase(tt)
        if prev is not None:
            combine_phase(*prev)
        w1, w2 = select_phase(a)
        prev = (tt, w1, w2)
    combine_phase(*prev)


def _postpool(nc, sb, sbd, ps, consts, x1t, x2t, py1, py2, out_dram, tag):
    """prepool conv + GN + relu + transpose + L2 normalize + store."""
    # bias = bp + Wp[:,144:] @ pooled_other
    pb = ps.tile([96, 1], f32, tag="sml")
    nc.tensor.matmul(pb, consts['wpt3'], py1, start=True, stop=False)
    nc.tensor.matmul(pb, consts['wpt4'], py2, start=False, stop=True)
    biascol = sb.tile([96, 1], f32, tag=f"biascol_{tag}")
    nc.vector.tensor_add(biascol, pb, consts['bp_col'])

    fpre = sb.tile([96, N], f32, tag=f"fpre_{tag}")
    for c in range(4):
        sl = slice(c * 512, (c + 1) * 512)
        p9 = ps.tile([96, 512], f32, tag="big")
        nc.tensor.matmul(p9, consts['wpt1'], x1t[:, sl], start=True, stop=False)
        nc.tensor.matmul(p9, consts['wpt2'], x2t[:, sl], start=False, stop=True)
        nc.vector.tensor_scalar_add(fpre[:, sl], p9, biascol[:, 0:1])

    # GroupNorm stats
    junk = sbd.tile([96, N], f16, tag="hcat")
    rowsum = sb.tile([96, 1], f32, tag=f"rowsum_{tag}")
    nc.scalar.activation(junk, fpre, Act.Copy, accum_out=rowsum)
    rowsq = sb.tile([96, 1], f32, tag=f"rowsq_{tag}")
    nc.scalar.activation(junk, fpre, Act.Square, accum_out=rowsq)

    # [96,1] -> [1,96] -> group [1,12] -> mu/var -> [12,1] -> [96,1]
    pt1 = ps.tile([1, 96], f32, tag="sml")
    nc.tensor.transpose(pt1, rowsum, consts['id_f32'][0:96, 0:96])
    sum_r = sb.tile([1, 96], f32, tag="sum_r")
    nc.vector.tensor_copy(sum_r, pt1)
    pt2 = ps.tile([1, 96], f32, tag="sml")
    nc.tensor.transpose(pt2, rowsq, consts['id_f32'][0:96, 0:96])
    sq_r = sb.tile([1, 96], f32, tag="sq_r")
    nc.vector.tensor_copy(sq_r, pt2)

    g12 = sb.tile([1, 12], f32, tag="g12")
    nc.vector.tensor_reduce(g12, sum_r[0:1, :].rearrange("a (g e) -> a g e", e=8),
                            axis=AxX, op=Alu.add)
    q12 = sb.tile([1, 12], f32, tag="q12")
    nc.vector.tensor_reduce(q12, sq_r[0:1, :].rearrange("a (g e) -> a g e", e=8),
                            axis=AxX, op=Alu.add)
    mu12 = sb.tile([1, 12], f32, tag="mu12")
    nc.vector.tensor_scalar_mul(mu12, g12, 1.0 / (8.0 * N))
    ex12 = sb.tile([1, 12], f32, tag="ex12")
    nc.vector.tensor_scalar_mul(ex12, q12, 1.0 / (8.0 * N))
    mu2 = sb.tile([1, 12], f32, tag="mu2")
    nc.vector.tensor_mul(mu2, mu12, mu12)
    var12 = sb.tile([1, 12], f32, tag="var12")
    nc.vector.tensor_sub(var12, ex12, mu2)
    nc.vector.tensor_scalar_add(var12, var12, GN_EPS)
    rec12 = sb.tile([1, 12], f32, tag="rec12")
    nc.vector.reciprocal(rec12, var12)
    rt12 = sb.tile([1, 12], f32, tag="rt12")
    nc.scalar.activation(rt12, rec12, Act.Sqrt)   # rsqrt(var+eps)

    pm = ps.tile([12, 1], f32, tag="sml")
    nc.tensor.transpose(pm, mu12, consts['id_f32'][0:1, 0:1])
    mucol12 = sb.tile([12, 1], f32, tag="mucol12")
    nc.vector.tensor_copy(mucol12, pm)
    pv = ps.tile([12, 1], f32, tag="sml")
    nc.tensor.transpose(pv, rt12, consts['id_f32'][0:1, 0:1])
    rtcol12 = sb.tile([12, 1], f32, tag="rtcol12")
    nc.vector.tensor_copy(rtcol12, pv)

    pmu96 = ps.tile([96, 1], f32, tag="sml")
    nc.tensor.matmul(pmu96, consts['gmat'], mucol12, start=True, stop=True)
    prt96 = ps.tile([96, 1], f32, tag="sml")
    nc.tensor.matmul(prt96, consts['gmat'], rtcol12, start=True, stop=True)

    acol = sb.tile([96, 1], f32, tag="acol")
    nc.vector.tensor_mul(acol, prt96, consts['gng_col'])
    tb = sb.tile([96, 1], f32, tag="tb")
    nc.vector.tensor_mul(tb, pmu96, acol)
    bcol = sb.tile([96, 1], f32, tag="bcol")
    nc.vector.tensor_sub(bcol, consts['gnb_col'], tb)

    nc.vector.tensor_scalar(fpre, fpre, acol[:, 0:1], scalar2=bcol[:, 0:1],
                            op0=Alu.mult, op1=Alu.add)

    # transpose to rows + relu, then L2 normalize, then store
    orow = sb.tile([128, NT, 96], f32, tag=f"orow_{tag}")
    for tt in range(NT):
        po = ps.tile([128, 96], f32, tag="sml")
        nc.tensor.transpose(po, fpre[:, tt * 128:(tt + 1) * 128],
                            consts['id_f32'][0:96, 0:96])
        nc.scalar.activation(orow[:, tt, :], po, Act.Relu)
        sc = sbd.tile([128, 96], f16, tag="sc")
        ssq = sbd.tile([128, 1], f32, tag="ssq")
        nc.scalar.activation(sc, orow[:, tt, :], Act.Square, accum_out=ssq)
        rs = sbd.tile([128, 1], f32, tag="rs")
        nc.vector.reciprocal(rs, ssq)
        nr = sbd.tile([128, 1], f32, tag="nr")
        nc.scalar.activation(nr, rs, Act.Sqrt)
        nc.vector.tensor_scalar_mul(orow[:, tt, :], orow[:, tt, :], nr[:, 0:1])
    nc.sync.dma_start(out=out_dram.rearrange("(t p) c -> p t c", p=128),
                      in_=orow[:])


def build():
    nc = bacc.Bacc("TRN2", target_bir_lowering=False, debug=False,
                   enable_asserts=True, num_devices=1)
    ptsx = nc.dram_tensor("ptsx", [N, 2], f32, kind="ExternalInput").ap()
    ptsy = nc.dram_tensor("ptsy", [N, 2], f32, kind="ExternalInput").ap()
    cdecl = {
        'ones_dram': ([1, N], f32),
        'lhs3_d': ([3, 48], f32),
        'lhs49_d': ([49, 96], f32),
        'id_f32_d': ([128, 128], f32),
        'id_f16_d': ([128, 128], f16),
        'offs_d': ([128, NCAND], f32),
        'offs16_d': ([128, NCAND], i16),
        'm1w_d': ([128, K], f16),
        'm2w_d': ([128, K], f16),
        'm1b_d': ([48, 1], f32),
        'm2b_d': ([96, 1], f32),
        'wpt1_d': ([48, 96], f32),
        'wpt2_d': ([96, 96], f32),
        'wpt3_d': ([48, 96], f32),
        'wpt4_d': ([96, 96], f32),
        'bp_d': ([96, 1], f32),
        'gng_d': ([96, 1], f32),
        'gnb_d': ([96, 1], f32),
        'gmat_d': ([12, 96], f32),
    }
    dram = {k: nc.dram_tensor(k, shp, dt, kind="ExternalInput").ap()
            for k, (shp, dt) in cdecl.items()}
    fx = nc.dram_tensor("fx", [N, 96], f32, kind="ExternalOutput").ap()
    fy = nc.dram_tensor("fy", [N, 96], f32, kind="ExternalOutput").ap()

    with TileContext(nc) as tc:
        with (
            tc.tile_pool(name="cpool", bufs=1) as cp,
            tc.tile_pool(name="sb1", bufs=1) as sb1,
            tc.tile_pool(name="sbd", bufs=2) as sbd,
            tc.tile_pool(name="ps", bufs=2, space="PSUM") as ps,
            tc.tile_pool(name="psb", bufs=1, space="PSUM") as psb,
        ):
            consts = {}
            for name, key in (('lhs3', 'lhs3_d'), ('lhs49', 'lhs49_d'),
                              ('id_f32', 'id_f32_d'), ('id_f16', 'id_f16_d'),
                              ('offs', 'offs_d'), ('offs16', 'offs16_d'), ('m1w_row', 'm1w_d'),
                              ('m2w_row', 'm2w_d'), ('m1b_col', 'm1b_d'),
                              ('m2b_col', 'm2b_d'), ('wpt1', 'wpt1_d'),
                              ('wpt2', 'wpt2_d'), ('wpt3', 'wpt3_d'),
                              ('wpt4', 'wpt4_d'), ('bp_col', 'bp_d'),
                              ('gng_col', 'gng_d'), ('gnb_col', 'gnb_d'),
                              ('gmat', 'gmat_d')):
                shp, dt = cdecl[key]
                tile = cp.tile(shp, dt, tag=name)
                nc.sync.dma_start(out=tile[:], in_=dram[key][:])
                consts[name] = tile
            consts['ones_dram'] = dram['ones_dram']
            e01 = cp.tile([1, 2], f32, tag="e01")
            nc.vector.memset(e01[:, 0:1], 1.0)
            nc.vector.memset(e01[:, 1:2], 0.0)
            e10 = cp.tile([1, 2], f32, tag="e10")
            nc.vector.memset(e10[:, 0:1], 0.0)
            nc.vector.memset(e10[:, 1:2], 1.0)
            consts['e01'], consts['e10'] = e01, e10

            xts = {}
            for tag, pts in (('bx', ptsx), ('by', ptsy)):
                x1t = cp.tile([48, N], f32, tag=f"x1t_{tag}")
                x2t = cp.tile([96, N], f32, tag=f"x2t_{tag}")
                br = _branch_phase12(nc, sb1, sbd, ps, pts, consts, tag)
                _branch_main(nc, sb1, sbd, ps, psb, br, consts, x1t, x2t)
                xts[tag] = (x1t, x2t)

            pools = {}
            for tag in ('bx', 'by'):
                x1t, x2t = xts[tag]
                p1 = cp.tile([48, 1], f32, tag=f"p1_{tag}")
                p2 = cp.tile([96, 1], f32, tag=f"p2_{tag}")
                nc.vector.tensor_reduce(p1, x1t, axis=AxX, op=Alu.max)
                nc.vector.tensor_reduce(p2, x2t, axis=AxX, op=Alu.max)
                pools[tag] = (p1, p2)

            _postpool(nc, sb1, sbd, ps, consts, xts['bx'][0], xts['bx'][1],
                      pools['by'][0], pools['by'][1], fx, 'bx')
            _postpool(nc, sb1, sbd, ps, consts, xts['by'][0], xts['by'][1],
                      pools['bx'][0], pools['bx'][1], fy, 'by')
    nc.compile()
    return nc


CHW_HOST = CHW


def _host_consts(W1, bn1_g, bn1_b, bn1_m, bn1_v, m1w, m1b,
                 W2, bn2_g, bn2_b, bn2_m, bn2_v, m2w, m2b,
                 Wp, bp, gn_g, gn_b):
    f = np.float32
    s1 = (bn1_g.astype(np.float64) / np.sqrt(bn1_v.astype(np.float64) + BN_EPS))
    A1 = (s1[:, None] * W1.astype(np.float64)).astype(f)          # (48, 2)
    c1 = (bn1_b.astype(np.float64) - bn1_m.astype(np.float64) * s1).astype(f)
    s2 = (bn2_g.astype(np.float64) / np.sqrt(bn2_v.astype(np.float64) + BN_EPS))
    A2 = (s2[:, None] * W2.astype(np.float64)).astype(f)          # (96, 48)
    c2 = (bn2_b.astype(np.float64) - bn2_m.astype(np.float64) * s2).astype(f)

    lhs3 = np.stack([A1[:, 0], A1[:, 1], c1], axis=0).astype(f)   # (3, 48)
    lhs49 = np.concatenate([A2.T, c2[None, :]], axis=0).astype(f)  # (49, 96)
    gmat = np.zeros((12, 96), f)
    for g in range(12):
        gmat[g, g * 8:(g + 1) * 8] = 1.0
    offs = (CHW * (np.arange(NCAND) // 16)).astype(f)
    c = {
        'ones_dram': np.ones((1, N), f),
        'lhs3_d': lhs3,
        'lhs49_d': lhs49,
        'id_f32_d': np.eye(128, dtype=f),
        'id_f16_d': np.eye(128, dtype=np.float16),
        'offs_d': np.broadcast_to(offs, (128, NCAND)).copy(),
        'offs16_d': np.broadcast_to(offs.astype(np.int16), (128, NCAND)).copy(),
        'm1w_d': np.broadcast_to(m1w.astype(np.float16), (128, K)).copy(),
        'm2w_d': np.broadcast_to(m2w.astype(np.float16), (128, K)).copy(),
        'm1b_d': np.full((48, 1), m1b[0], f),
        'm2b_d': np.full((96, 1), m2b[0], f),
        'wpt1_d': Wp[:, 0:48].T.astype(f).copy(),
        'wpt2_d': Wp[:, 48:144].T.astype(f).copy(),
        'wpt3_d': Wp[:, 144:192].T.astype(f).copy(),
        'wpt4_d': Wp[:, 192:288].T.astype(f).copy(),
        'bp_d': bp.reshape(96, 1).astype(f),
        'gng_d': gn_g.reshape(96, 1).astype(f),
        'gnb_d': gn_b.reshape(96, 1).astype(f),
        'gmat_d': gmat,
    }
    return c


def kernel(x, y, W1, bn1_g, bn1_b, bn1_m, bn1_v, m1w, m1b,
           W2, bn2_g, bn2_b, bn2_m, bn2_v, m2w, m2b, Wp, bp, gn_g, gn_b):
    x = np.ascontiguousarray(np.asarray(x, np.float32))
    y = np.ascontiguousarray(np.asarray(y, np.float32))
    if 'nc' not in _CACHED:
        _CACHED['nc'] = build()
    nc = _CACHED['nc']
    consts = _host_consts(W1, bn1_g, bn1_b, bn1_m, bn1_v, m1w, m1b,
                          W2, bn2_g, bn2_b, bn2_m, bn2_v, m2w, m2b,
                          Wp, bp, gn_g, gn_b)
    B = x.shape[0]
    in_maps = []
    for b in range(B):
        m = {'ptsx': x[b], 'ptsy': y[b]}
        m.update(consts)
        in_maps.append(m)
    res = run_bass_kernel_spmd(nc, in_maps, list(range(B)))
    fx = np.stack([res.results[b]['fx'] for b in range(B)])
    fy = np.stack([res.results[b]['fy'] for b in range(B)])
    return fx, fy


if __name__ == '__main__':
    Z = np.load('/tmp/inputs.npz')
    out = kernel(**{k: Z[k] for k in Z.files})
    print(out[0].shape, out[1].shape)



# revision 19
# speedup vs baseline: 1.6213x; 1.6213x over previous
"""DGCNN-style kernel for Trainium2 (8 NeuronCores, data-parallel over batch).

Per core: one batch sample, both branches (x, y).
Pipeline per branch:
  1. A[n,j] = 2 x_n.x_j + 2 y_n.y_j - sq_j (PE f32r 4-row matmul) - sq_n
     (ACT bias on the PSUM->SBUF copy).  A == -(pairwise distance).
  2. top-32 select: per 128-wide chunk top-8 (max8 + max_index), then
     top-32 merge of the 128 candidates (max8/max_index/match_replace).
  3. rank weights scattered into candidate space (local_scatter by merge
     position), then per-chunk [128,128] weight tiles scattered directly
     (chunk == j-block, so no global index fixup).
  4. weight tiles transposed on PE into PSUM (8 per bank), copied to SBUF
     in 1024-wide batches (ACT for w1, GPSIMD for w2).
  5. X1 = H1 @ W1^T, X2 = H2 @ W2^T accumulated on PE (f16).
  6. cross-branch max-pool, 288->96 conv (pooled part folded into bias),
     GroupNorm(12), relu, transpose, row L2-normalize.
Tiles are software-pipelined 3 deep: A-build(t+2) | select(t+1) | combine(t).
"""
import sys

sys.path.insert(0, '/opt/trn_rl_repo')
sys.path.insert(0, '/opt/pypackages')

import numpy as np
import concourse.bacc as bacc
import concourse.mybir as mybir
from concourse.tile import TileContext
from concourse.bass_utils import run_bass_kernel_spmd

N = 2048
K = 32
NT = N // 128          # 16 n-tiles / j-blocks
NCH = 16               # chunks per row (chunk == j-block)
CHW = N // NCH         # 128 chunk width
NCAND = NCH * 8        # 128 candidates per row
BN_EPS = 1e-5
GN_EPS = 1e-5
NEG = -1.0e9

f32 = mybir.dt.float32
f32r = mybir.dt.float32r
f16 = mybir.dt.float16
u16 = mybir.dt.uint16
i16 = mybir.dt.int16
Alu = mybir.AluOpType
Act = mybir.ActivationFunctionType
AxX = mybir.AxisListType.X

_CACHED = {}


def _phase1(nc, sb, sbd, sbp, ps, pss, pts, consts, tag):
    """Load pts, build rows4r/rows_str (f32r), H1e/H2T, Hcat. Returns dict."""
    # stage rows assembled by DMA (f32), then one ACT pass each -> f32r
    # rows4 = [x; y; -sq; ones] (moving), rows_st = [2x; 2y; ones; -sq] (stat)
    rows4 = sb.tile([4, N], f32, tag=f"rows4_{tag}")
    nc.sync.dma_start(out=rows4[0:2, :], in_=pts.rearrange("(a n) c -> (a c) n", a=1))
    nc.sync.dma_start(out=rows4[3:4, :], in_=consts['onz_dram'][0:1, :])

    # nsq columns [128, NT] = -(x^2 + y^2); -sq row via PE transpose + DRAM
    ccols = sbd.tile([128, NT, 2], f32, tag="ccols")
    nc.sync.dma_start(out=ccols[:], in_=pts.rearrange("(t p) c -> p t c", p=128))
    t1 = sbd.tile([128, NT], f32, tag="t1")
    nc.vector.tensor_mul(t1, ccols[:, :, 0], ccols[:, :, 0])
    t2 = sbd.tile([128, NT], f32, tag="t2")
    nc.vector.tensor_mul(t2, ccols[:, :, 1], ccols[:, :, 1])
    nsq = sb.tile([128, NT], f32, tag=f"nsq_{tag}")
    nc.vector.scalar_tensor_tensor(nsq, t1, -1.0, t2,
                                   op0=Alu.mult, op1=Alu.subtract)
    pqt = pss.tile([NT, 128], f32, tag="s")
    nc.tensor.transpose(pqt, nsq, consts['id_f32'][0:128, 0:128])
    sqt = sbd.tile([NT, 128], f32, tag="sqt")
    nc.vector.tensor_copy(sqt, pqt)
    sq_dram = nc.dram_tensor(f"sq_dram_{tag}", [1, N], f32)
    nc.sync.dma_start(out=sq_dram[0:1, :].rearrange("a (t p) -> (a t) p", p=128),
                      in_=sqt[:])
    nc.sync.dma_start(out=rows4[2:3, :], in_=sq_dram[0:1, :])

    rows_st = sb.tile([4, N], f32, tag=f"rowsst_{tag}")
    nc.scalar.activation(rows_st[0:2, :], rows4[0:2, :], Act.Identity, scale=2.0)
    nc.sync.dma_start(out=rows_st[2:3, :], in_=consts['onz_dram'][0:1, :])
    nc.sync.dma_start(out=rows_st[3:4, :], in_=sq_dram[0:1, :])

    # H1e [48, N] f32r : relu(A1 p + c1)
    h1e = sbp.tile([48, N], f32, tag="h1e")
    for c in range(4):
        sl = slice(c * 512, (c + 1) * 512)
        ph = pss.tile([48, 512], f32, tag="s")
        nc.tensor.matmul(ph, consts['lhs4'],
                         rows4[:, sl], start=True, stop=True)
        nc.scalar.activation(h1e[:, sl], ph, Act.Relu)

    # H2T [96, N] f32 : relu(A2 H1 + c2), c2 as ACT bias
    h2t = sbp.tile([96, N], f32, tag="h2t")
    for c in range(4):
        sl = slice(c * 512, (c + 1) * 512)
        ph2 = pss.tile([96, 512], f32, tag="s")
        nc.tensor.matmul(ph2, consts['lhs48'],
                         h1e[:, sl], start=True, stop=True)
        nc.scalar.activation(h2t[:, sl], ph2, Act.Relu,
                             bias=consts['c2_col'][:, 0:1], scale=1.0)

    # Hcat [128, NT, 144] f16 via PE transposes, one ACT copy per j-block
    hcat = sb.tile([128, NT, 144], f16, tag=f"hcat_{tag}")
    for jb in range(NT):
        jsl = slice(jb * 128, (jb + 1) * 128)
        pc = pss.tile([128, 144], f32, tag="s")
        nc.tensor.transpose(pc[:, 0:48], h1e[:, jsl],
                            consts['id_f32'][0:48, 0:48])
        nc.tensor.transpose(pc[:, 48:144], h2t[:, jsl], consts['id_f32'][0:96, 0:96])
        nc.scalar.activation(hcat[:, jb, :], pc, Act.Identity)

    return dict(rows4=rows4, rows_st=rows_st, hcat=hcat)


def _branch_loop(nc, sb, sbd, sbw, ps, psw, psx, br, consts, x1t, x2t):
    """Per n-tile pipeline: A-build | top-32 select | scatter+combine."""
    rows4, rows_st, hcat = br['rows4'], br['rows_st'], br['hcat']
    a_t, sel_t, px_t = {}, {}, {}

    def emit_a(tt):
        nsl = slice(tt * 128, (tt + 1) * 128)
        a = sbd.tile([128, N], f32, tag="a")
        for h in range(2):
            pt = ps.tile([128, 1024], f32, tag="pt")
            for c in range(2):
                sl = slice(h * 1024 + c * 512, h * 1024 + (c + 1) * 512)
                nc.tensor.matmul(pt[:, c * 512:(c + 1) * 512],
                                 rows_st[:, nsl],
                                 rows4[:, sl],
                                 start=True, stop=True)
            nc.scalar.activation(a[:, h * 1024:(h + 1) * 1024], pt, Act.Identity)
        a_t[tt] = a

    def emit_select(tt):
        a = a_t.pop(tt)
        cv = sbd.tile([128, NCAND], f32, tag="cv")
        ci = sbd.tile([128, NCAND], u16, tag="ci")
        for c8 in range(NCH):
            s = slice(c8 * 8, c8 * 8 + 8)
            asl = a[:, c8 * CHW:(c8 + 1) * CHW]
            nc.vector.max(cv[:, s], asl)
            nc.vector.max_index(ci[:, s], cv[:, s], asl)
        mm = sbd.tile([128, K], f32, tag="mm")
        pp = sbd.tile([128, K], u16, tag="pp")
        for r in range(4):
            rsl = slice(r * 8, r * 8 + 8)
            if r:
                nc.vector.match_replace(cv, in_to_replace=mm[:, r * 8 - 8:r * 8],
                                        in_values=cv, imm_value=NEG)
            nc.vector.max(mm[:, rsl], cv)
            nc.vector.max_index(pp[:, rsl], mm[:, rsl], cv)
        cw1 = sbd.tile([128, NCAND], f16, tag="cw1")
        cw2 = sbd.tile([128, NCAND], f16, tag="cw2")
        ppi = pp[:].bitcast(i16)
        nc.gpsimd.local_scatter(cw1, consts['m1w_row'], ppi,
                                channels=128, num_elems=NCAND, num_idxs=K)
        nc.gpsimd.local_scatter(cw2, consts['m2w_row'], ppi,
                                channels=128, num_elems=NCAND, num_idxs=K)
        sel_t[tt] = (ci, cw1, cw2)

    def emit_combine(tt):
        ci, cw1, cw2 = sel_t.pop(tt)
        cii = ci[:].bitcast(i16)
        # px psum banks are shared by 4 consecutive tiles (batched ACT store)
        q = tt % 4
        if q == 0:
            pxa = psx.tile([48, 512], f32, tag="px1")
            pxb = psx.tile([96, 512], f32, tag="px2")
            px_t[0], px_t[1] = pxa, pxb
        px1 = px_t[0][:, q * 128:(q + 1) * 128]
        px2 = px_t[1][:, q * 128:(q + 1) * 128]
        wt1 = sbd.tile([128, N], f16, tag="wt1")
        wt2 = sbd.tile([128, N], f16, tag="wt2")
        for hh in range(2):
            hsl = slice(hh * 1024, (hh + 1) * 1024)
            for w, cw, wt in ((1, cw1, wt1), (2, cw2, wt2)):
                pw = psw.tile([128, 1024], f16, tag="pw")
                for j8 in range(8):
                    jb = hh * 8 + j8
                    js = slice(jb * 8, jb * 8 + 8)
                    wc = sbw.tile([128, 128], f16, tag="wc")
                    nc.gpsimd.local_scatter(wc, cw[:, js], cii[:, js],
                                            channels=128, num_elems=128,
                                            num_idxs=8)
                    nc.tensor.transpose(pw[:, j8 * 128:(j8 + 1) * 128], wc,
                                        consts['id_f16'])
                nc.scalar.activation(wt[:, hsl], pw, Act.Identity)
            for j8 in range(8):
                jb = hh * 8 + j8
                jsl = slice(jb * 128, (jb + 1) * 128)
                nc.tensor.matmul(px1, hcat[:, jb, 0:48], wt1[:, jsl],
                                 start=(jb == 0), stop=(jb == NT - 1))
                nc.tensor.matmul(px2, hcat[:, jb, 48:144], wt2[:, jsl],
                                 start=(jb == 0), stop=(jb == NT - 1))
        if q == 3:
            bsl = slice((tt - 3) * 128, (tt + 1) * 128)
            nc.scalar.activation(x1t[:, bsl], px_t[0], Act.Identity,
                                 bias=consts['m1b_col'][:, 0:1], scale=1.0)
            nc.scalar.activation(x2t[:, bsl], px_t[1], Act.Identity,
                                 bias=consts['m2b_col'][:, 0:1], scale=1.0)

    for it in range(NT + 2):
        if it < NT:
            emit_a(it)
        if 1 <= it <= NT:
            emit_select(it - 1)
        if it >= 2:
            emit_combine(it - 2)


def _postpool(nc, sb, sbd, ps, pss, consts, x1t, x2t, py1, py2, out_dram, tag):
    """prepool conv + GN + relu + transpose + L2 normalize + store."""
    # bias = bp + Wp[:,144:] @ pooled_other
    pb = pss.tile([96, 1], f32, tag="s")
    nc.tensor.matmul(pb, consts['wpt3'], py1, start=True, stop=False)
    nc.tensor.matmul(pb, consts['wpt4'], py2, start=False, stop=True)
    biascol = sb.tile([96, 1], f32, tag=f"biascol_{tag}")
    nc.vector.tensor_add(biascol, pb, consts['bp_col'])

    fpre = sb.tile([96, N], f32, tag=f"fpre_{tag}")
    for c in range(4):
        sl = slice(c * 512, (c + 1) * 512)
        p9 = pss.tile([96, 512], f32, tag="s")
        nc.tensor.matmul(p9, consts['wpt1'],
                         x1t[:, sl], start=True, stop=False)
        nc.tensor.matmul(p9, consts['wpt2'],
                         x2t[:, sl], start=False, stop=True)
        nc.vector.tensor_scalar_add(fpre[:, sl], p9, biascol[:, 0:1])

    # GroupNorm stats
    junk = sbd.tile([96, N], f16, tag="junk")
    rowsum = sb.tile([96, 1], f32, tag=f"rowsum_{tag}")
    nc.scalar.activation(junk, fpre, Act.Copy, accum_out=rowsum)
    rowsq = sb.tile([96, 1], f32, tag=f"rowsq_{tag}")
    nc.scalar.activation(junk, fpre, Act.Square, accum_out=rowsq)

    # [96,1] -> [1,96] -> group [1,12] -> mu/var -> [12,1] -> [96,1]
    pt1 = pss.tile([1, 96], f32, tag="s")
    nc.tensor.transpose(pt1, rowsum, consts['id_f32'][0:96, 0:96])
    sum_r = sb.tile([1, 96], f32, tag="sum_r")
    nc.vector.tensor_copy(sum_r, pt1)
    pt2 = pss.tile([1, 96], f32, tag="s")
    nc.tensor.transpose(pt2, rowsq, consts['id_f32'][0:96, 0:96])
    sq_r = sb.tile([1, 96], f32, tag="sq_r")
    nc.vector.tensor_copy(sq_r, pt2)

    g12 = sb.tile([1, 12], f32, tag="g12")
    nc.vector.tensor_reduce(g12, sum_r[0:1, :].rearrange("a (g e) -> a g e", e=8),
                            axis=AxX, op=Alu.add)
    q12 = sb.tile([1, 12], f32, tag="q12")
    nc.vector.tensor_reduce(q12, sq_r[0:1, :].rearrange("a (g e) -> a g e", e=8),
                            axis=AxX, op=Alu.add)
    mu12 = sb.tile([1, 12], f32, tag="mu12")
    nc.vector.tensor_scalar_mul(mu12, g12, 1.0 / (8.0 * N))
    ex12 = sb.tile([1, 12], f32, tag="ex12")
    nc.vector.tensor_scalar_mul(ex12, q12, 1.0 / (8.0 * N))
    mu2 = sb.tile([1, 12], f32, tag="mu2")
    nc.vector.tensor_mul(mu2, mu12, mu12)
    var12 = sb.tile([1, 12], f32, tag="var12")
    nc.vector.tensor_sub(var12, ex12, mu2)
    nc.vector.tensor_scalar_add(var12, var12, GN_EPS)
    rec12 = sb.tile([1, 12], f32, tag="rec12")
    nc.vector.reciprocal(rec12, var12)
    rt12 = sb.tile([1, 12], f32, tag="rt12")
    nc.scalar.activation(rt12, rec12, Act.Sqrt)   # rsqrt(var+eps)

    pm = pss.tile([12, 1], f32, tag="s")
    nc.tensor.transpose(pm, mu12, consts['id_f32'][0:1, 0:1])
    mucol12 = sb.tile([12, 1], f32, tag="mucol12")
    nc.vector.tensor_copy(mucol12, pm)
    pv = pss.tile([12, 1], f32, tag="s")
    nc.tensor.transpose(pv, rt12, consts['id_f32'][0:1, 0:1])
    rtcol12 = sb.tile([12, 1], f32, tag="rtcol12")
    nc.vector.tensor_copy(rtcol12, pv)

    pmu96 = pss.tile([96, 1], f32, tag="s")
    nc.tensor.matmul(pmu96, consts['gmat'], mucol12, start=True, stop=True)
    prt96 = pss.tile([96, 1], f32, tag="s")
    nc.tensor.matmul(prt96, consts['gmat'], rtcol12, start=True, stop=True)

    acol = sb.tile([96, 1], f32, tag="acol")
    nc.vector.tensor_mul(acol, prt96, consts['gng_col'])
    tb = sb.tile([96, 1], f32, tag="tb")
    nc.vector.tensor_mul(tb, pmu96, acol)
    bcol = sb.tile([96, 1], f32, tag="bcol")
    nc.vector.tensor_sub(bcol, consts['gnb_col'], tb)

    nc.vector.tensor_scalar(fpre, fpre, acol[:, 0:1], scalar2=bcol[:, 0:1],
                            op0=Alu.mult, op1=Alu.add)

    # transpose to rows + relu, then L2 normalize, then store
    orow = sb.tile([128, NT, 96], f32, tag=f"orow_{tag}")
    for tt in range(NT):
        po = pss.tile([128, 96], f32, tag="s")
        nc.tensor.transpose(po, fpre[:, tt * 128:(tt + 1) * 128],
                            consts['id_f32'][0:96, 0:96])
        nc.scalar.activation(orow[:, tt, :], po, Act.Relu)
        sc = sbd.tile([128, 96], f16, tag="sc")
        ssq = sbd.tile([128, 1], f32, tag="ssq")
        nc.scalar.activation(sc, orow[:, tt, :], Act.Square, accum_out=ssq)
        rs = sbd.tile([128, 1], f32, tag="rs")
        nc.vector.reciprocal(rs, ssq)
        nr = sbd.tile([128, 1], f32, tag="nr")
        nc.scalar.activation(nr, rs, Act.Sqrt)
        nc.vector.tensor_scalar_mul(orow[:, tt, :], orow[:, tt, :], nr[:, 0:1])
    nc.sync.dma_start(out=out_dram.rearrange("(t p) c -> p t c", p=128),
                      in_=orow[:])


def build():
    nc = bacc.Bacc("TRN2", target_bir_lowering=False, debug=False,
                   enable_asserts=True, num_devices=1)
    ptsx = nc.dram_tensor("ptsx", [N, 2], f32, kind="ExternalInput").ap()
    ptsy = nc.dram_tensor("ptsy", [N, 2], f32, kind="ExternalInput").ap()
    cdecl = {
        'onz_dram': ([2, N], f32),
        'lhs4_d': ([4, 48], f32),
        'lhs48_d': ([48, 96], f32),
        'c2_d': ([96, 1], f32),
        'id_f32_d': ([128, 128], f32),
        'id_f16_d': ([128, 128], f16),
        'm1w_d': ([128, K], f16),
        'm2w_d': ([128, K], f16),
        'm1b_d': ([48, 1], f32),
        'm2b_d': ([96, 1], f32),
        'wpt1_d': ([48, 96], f32),
        'wpt2_d': ([96, 96], f32),
        'wpt3_d': ([48, 96], f32),
        'wpt4_d': ([96, 96], f32),
        'bp_d': ([96, 1], f32),
        'gng_d': ([96, 1], f32),
        'gnb_d': ([96, 1], f32),
        'gmat_d': ([12, 96], f32),
    }
    dram = {k: nc.dram_tensor(k, shp, dt, kind="ExternalInput").ap()
            for k, (shp, dt) in cdecl.items()}
    fx = nc.dram_tensor("fx", [N, 96], f32, kind="ExternalOutput").ap()
    fy = nc.dram_tensor("fy", [N, 96], f32, kind="ExternalOutput").ap()

    with TileContext(nc) as tc:
        with (
            tc.tile_pool(name="cpool", bufs=1) as cp,
            tc.tile_pool(name="sb1", bufs=1) as sb1,
            tc.tile_pool(name="sbd", bufs=2) as sbd,
            tc.tile_pool(name="sbp", bufs=1) as sbp,
            tc.tile_pool(name="sbw", bufs=4) as sbw,
            tc.tile_pool(name="ps", bufs=1, space="PSUM") as ps,
            tc.tile_pool(name="psw", bufs=2, space="PSUM") as psw,
            tc.tile_pool(name="psx", bufs=1, space="PSUM") as psx,
            tc.tile_pool(name="pss", bufs=2, space="PSUM") as pss,
        ):
            consts = {}
            for name, key in (('lhs4', 'lhs4_d'), ('lhs48', 'lhs48_d'),
                              ('c2_col', 'c2_d'),
                              ('id_f32', 'id_f32_d'), ('id_f16', 'id_f16_d'),
                              ('m1w_row', 'm1w_d'), ('m2w_row', 'm2w_d'),
                              ('m1b_col', 'm1b_d'), ('m2b_col', 'm2b_d'),
                              ('wpt1', 'wpt1_d'), ('wpt2', 'wpt2_d'),
                              ('wpt3', 'wpt3_d'), ('wpt4', 'wpt4_d'),
                              ('bp_col', 'bp_d'), ('gng_col', 'gng_d'),
                              ('gnb_col', 'gnb_d'), ('gmat', 'gmat_d')):
                shp, dt = cdecl[key]
                tile = cp.tile(shp, dt, tag=name)
                nc.sync.dma_start(out=tile[:], in_=dram[key][:])
                consts[name] = tile
            consts['onz_dram'] = dram['onz_dram']

            xts = {}
            for tag, pts in (('bx', ptsx), ('by', ptsy)):
                x1t = cp.tile([48, N], f32, tag=f"x1t_{tag}")
                x2t = cp.tile([96, N], f32, tag=f"x2t_{tag}")
                br = _phase1(nc, sb1, sbd, sbp, ps, pss, pts, consts, tag)
                _branch_loop(nc, sb1, sbd, sbw, ps, psw, psx, br, consts,
                             x1t, x2t)
                xts[tag] = (x1t, x2t)

            pools = {}
            for tag in ('bx', 'by'):
                x1t, x2t = xts[tag]
                p1 = cp.tile([48, 1], f32, tag=f"p1_{tag}")
                p2 = cp.tile([96, 1], f32, tag=f"p2_{tag}")
                nc.vector.tensor_reduce(p1, x1t, axis=AxX, op=Alu.max)
                nc.vector.tensor_reduce(p2, x2t, axis=AxX, op=Alu.max)
                pools[tag] = (p1, p2)

            _postpool(nc, sb1, sbd, ps, pss, consts, xts['bx'][0], xts['bx'][1],
                      pools['by'][0], pools['by'][1], fx, 'bx')
            _postpool(nc, sb1, sbd, ps, pss, consts, xts['by'][0], xts['by'][1],
                      pools['bx'][0], pools['bx'][1], fy, 'by')
    nc.compile()
    return nc


def _host_consts(W1, bn1_g, bn1_b, bn1_m, bn1_v, m1w, m1b,
                 W2, bn2_g, bn2_b, bn2_m, bn2_v, m2w, m2b,
                 Wp, bp, gn_g, gn_b):
    f = np.float32
    s1 = (bn1_g.astype(np.float64) / np.sqrt(bn1_v.astype(np.float64) + BN_EPS))
    A1 = (s1[:, None] * W1.astype(np.float64)).astype(f)          # (48, 2)
    c1 = (bn1_b.astype(np.float64) - bn1_m.astype(np.float64) * s1).astype(f)
    s2 = (bn2_g.astype(np.float64) / np.sqrt(bn2_v.astype(np.float64) + BN_EPS))
    A2 = (s2[:, None] * W2.astype(np.float64)).astype(f)          # (96, 48)
    c2 = (bn2_b.astype(np.float64) - bn2_m.astype(np.float64) * s2).astype(f)

    # rows4 = [x; y; -sq; ones] -> lhs4 rows [A1x; A1y; 0; c1]
    lhs4 = np.stack([A1[:, 0], A1[:, 1], np.zeros(48, f), c1], axis=0).astype(f)
    lhs48 = A2.T.astype(f).copy()                                  # (48, 96)
    gmat = np.zeros((12, 96), f)
    for g in range(12):
        gmat[g, g * 8:(g + 1) * 8] = 1.0
    onz = np.zeros((2, N), f)
    onz[0, :] = 1.0
    c = {
        'onz_dram': onz,
        'lhs4_d': lhs4,
        'lhs48_d': lhs48,
        'c2_d': c2.reshape(96, 1).astype(f),
        'id_f32_d': np.eye(128, dtype=f),
        'id_f16_d': np.eye(128, dtype=np.float16),
        'm1w_d': np.broadcast_to(m1w.astype(np.float16), (128, K)).copy(),
        'm2w_d': np.broadcast_to(m2w.astype(np.float16), (128, K)).copy(),
        'm1b_d': np.full((48, 1), m1b[0], f),
        'm2b_d': np.full((96, 1), m2b[0], f),
        'wpt1_d': Wp[:, 0:48].T.astype(f).copy(),
        'wpt2_d': Wp[:, 48:144].T.astype(f).copy(),
        'wpt3_d': Wp[:, 144:192].T.astype(f).copy(),
        'wpt4_d': Wp[:, 192:288].T.astype(f).copy(),
        'bp_d': bp.reshape(96, 1).astype(f),
        'gng_d': gn_g.reshape(96, 1).astype(f),
        'gnb_d': gn_b.reshape(96, 1).astype(f),
        'gmat_d': gmat,
    }
    return c


def kernel(x, y, W1, bn1_g, bn1_b, bn1_m, bn1_v, m1w, m1b,
           W2, bn2_g, bn2_b, bn2_m, bn2_v, m2w, m2b, Wp, bp, gn_g, gn_b):
    x = np.ascontiguousarray(np.asarray(x, np.float32))
    y = np.ascontiguousarray(np.asarray(y, np.float32))
    if 'nc' not in _CACHED:
        _CACHED['nc'] = build()
    nc = _CACHED['nc']
    consts = _host_consts(W1, bn1_g, bn1_b, bn1_m, bn1_v, m1w, m1b,
                          W2, bn2_g, bn2_b, bn2_m, bn2_v, m2w, m2b,
                          Wp, bp, gn_g, gn_b)
    B = x.shape[0]
    in_maps = []
    for b in range(B):
        m = {'ptsx': x[b], 'ptsy': y[b]}
        m.update(consts)
        in_maps.append(m)
    res = run_bass_kernel_spmd(nc, in_maps, list(range(B)))
    fx = np.stack([res.results[b]['fx'] for b in range(B)])
    fy = np.stack([res.results[b]['fy'] for b in range(B)])
    return fx, fy


if __name__ == '__main__':
    Z = np.load('/tmp/inputs.npz')
    out = kernel(**{k: Z[k] for k in Z.files})
    print(out[0].shape, out[1].shape)
